# revision 31
# baseline (speedup 1.0000x reference)
"""Multi-Head Latent Attention (MLA) forward on 8 Trainium2 NeuronCores.

Sharding: tensor-parallel over heads (16 heads -> 2 per core). Each core:
  - transposes x to feature-major layout on the PE (fp32r),
  - computes q / latent-kv projections for all tokens (wkv_a replicated),
  - RMS-norms the latent (norm weight folded into wkv_b on host, per-token
    scale applied after the up-projection),
  - applies RoPE with host-precomputed cos/sin tables,
  - runs causal attention for its 2 heads in transposed-score layout
    (scores St[k, q]; softmax without max subtraction - scores are O(1)),
  - AllToAll exchanges per-head outputs so each core holds all features for
    a 512-token slice, then computes that slice of the wo projection.
Output slices are disjoint; the host just concatenates them.
"""
import sys

if "/opt/trn_rl_repo" not in sys.path:
    sys.path.insert(0, "/opt/trn_rl_repo")

import numpy as np
import concourse.bacc as bacc
import concourse.mybir as mybir
from concourse import tile
from concourse.masks import make_identity
from concourse.bass_utils import run_bass_kernel_spmd

H, NOPE, ROPE, VD, KVR, QKD = 16, 128, 64, 128, 512, 192
B, T, D = 2, 2048, 2048
NCORES, HPC, BLK = 8, 2, 512
W1N = HPC * QKD + KVR + ROPE  # 960 projection columns per core
f32 = mybir.dt.float32
f32r = mybir.dt.float32r
EXP = mybir.ActivationFunctionType.Exp
LN = mybir.ActivationFunctionType.Ln


def r32(ap):
    return ap.bitcast(f32r)


def _patch_act_tables():
    """Make the act-table-load pass serve Exp AND Ln from the one set that
    contains both (natural_log_exp_and_others), so interleaved exp/ln
    activations don't thrash table loads. Indices into act_info.json must be
    preserved, so the single-function sets are emptied in place, not removed.
    """
    import concourse.bacc as _bacc

    orig = _bacc.get_activation_tables
    if getattr(_bacc, "_mla_act_patch", False):
        return
    _bacc._mla_act_patch = True

    def patched(arch):
        d = dict(orig(arch))
        if "natural_log_exp_and_others" in d:
            for name in ("exp_and_others", "natural_log", "exp_and_friends"):
                if name in d:
                    d[name] = set()
        return d

    _bacc.get_activation_tables = patched


def build_program():
    _patch_act_tables()
    nc = bacc.Bacc("TRN2", target_bir_lowering=False, debug=False, num_devices=NCORES)
    x_d = nc.dram_tensor("x", [B * T, D], f32, kind="ExternalInput")
    w1_d = nc.dram_tensor("w1", [D, W1N], f32, kind="ExternalInput")
    wb_d = nc.dram_tensor("wb", [KVR, HPC * (NOPE + VD)], f32, kind="ExternalInput")
    wo_d = nc.dram_tensor("wo", [H * VD, D], f32, kind="ExternalInput")
    cos_d = nc.dram_tensor("cos", [128, T], f32, kind="ExternalInput")
    sin_d = nc.dram_tensor("sin", [128, T], f32, kind="ExternalInput")
    out_d = nc.dram_tensor("out", [B, T // NCORES, D], f32, kind="ExternalOutput")

    with tile.TileContext(nc) as tc:
        with tc.tile_pool(name="dram", bufs=1, space="DRAM") as dram:
            y_in = [
                dram.tile([NCORES, HPC * VD, 256], f32, name=f"y_in{b}")
                for b in range(B)
            ]
            y_out = [
                dram.tile([NCORES, HPC * VD, 256], f32, name=f"y_out{b}")
                for b in range(B)
            ]

            _phase1(nc, tc, x_d, w1_d, wb_d, cos_d, sin_d, y_in, y_out)

            _phase2_wo(nc, tc, y_out, wo_d, out_d)

    nc.compile()
    return nc


def _phase1(nc, tc, x_d, w1_d, wb_d, cos_d, sin_d, y_in, y_out):
    with (
        tc.tile_pool(name="const", bufs=1) as const,
        tc.tile_pool(name="wpool", bufs=1) as wpool,
        tc.tile_pool(name="kvpool", bufs=1) as kvpool,
        tc.tile_pool(name="xtpool", bufs=1) as xtpool,
        tc.tile_pool(name="xnpool", bufs=4) as xnpool,
        tc.tile_pool(name="work", bufs=1) as work,
        tc.tile_pool(name="ps", bufs=1, space="PSUM") as ps,
    ):
        ident_f = const.tile([128, 128], f32, tag="ident_f")
        make_identity(nc, ident_f)
        ident = const.tile([128, 128], f32r, tag="ident")
        nc.vector.tensor_copy(ident[:], ident_f[:])
        ones_f = const.tile([128, 1], f32, tag="ones_f")
        nc.gpsimd.memset(ones_f[:], 1.0)
        ones = const.tile([128, 1], f32r, tag="ones")
        nc.vector.tensor_copy(ones[:], ones_f[:])
        onesrow_f = const.tile([1, 128], f32, tag="onesrow_f")
        nc.gpsimd.memset(onesrow_f[:], 1.0)
        onesrow = const.tile([1, 128], f32r, tag="onesrow")
        nc.vector.tensor_copy(onesrow[:], onesrow_f[:])
        eps = const.tile([1, 1], f32, tag="eps")
        nc.gpsimd.memset(eps[:], 1e-6)
        w1_sb = wpool.tile([128, 16, W1N], f32r, tag="w1")
        nc.sync.dma_start(w1_sb[:], w1_d[:].rearrange("(kc p) m -> p kc m", p=128).bitcast(f32r))
        wb_sb = wpool.tile([128, 4, 512], f32r, tag="wb")
        nc.sync.dma_start(wb_sb[:], wb_d[:].rearrange("(kc p) m -> p kc m", p=128).bitcast(f32r))

        def stage_a(b, qc, knope, vnat, krope):
            """DMA + transpose + projections + rope + rms + kv for one block."""
            row0 = b * T + qc * BLK
            tok = slice(qc * BLK, (qc + 1) * BLK)
            if True:
                csb = work.tile([128, BLK], f32, tag="csb", bufs=1)
                ssb = work.tile([128, BLK], f32, tag="ssb", bufs=1)
                nc.sync.dma_start(csb[:], cos_d[:, tok])
                nc.sync.dma_start(ssb[:], sin_d[:, tok])

                # ---- transpose x block to feature-major xT (2048 x 512) ----
                xT = xtpool.tile([128, 16, BLK], f32r, tag="xT", bufs=1)
                for k4 in range(4):
                    xns = []
                    for tt in range(4):
                        xn = xnpool.tile([128, BLK], f32r, tag="xn", bufs=4)
                        nc.sync.dma_start(
                            xn[:],
                            x_d[row0 + tt * 128 : row0 + (tt + 1) * 128,
                                k4 * 512 : (k4 + 1) * 512].bitcast(f32r),
                        )
                        xns.append(xn)
                    for kk in range(4):
                        kc = k4 * 4 + kk
                        pst = ps.tile([128, BLK], f32, tag="xps", bufs=1)
                        for tt in range(4):
                            nc.tensor.transpose(
                                r32(pst[:, tt * 128 : (tt + 1) * 128]),
                                r32(xns[tt][:, kk * 128 : (kk + 1) * 128]),
                                r32(ident[:]),
                            )
                        nc.vector.tensor_copy(xT[:, kc, :], pst[:])

                # ---- fused projection: [q | latent | krope] = W1.T @ xT ----
                # two half-passes of 4 M-chunks to bound PSUM usage
                pmc = {}
                for half in range(2):
                    pp = [
                        ps.tile([128, BLK], f32, tag="proj", bufs=4, name="projp") for _ in range(4)
                    ]
                    for kc in range(16):
                        for i in range(4):
                            mc = half * 4 + i
                            m0 = mc * 128
                            m1 = min(m0 + 128, W1N)
                            nc.tensor.matmul(
                                pp[i][: m1 - m0, :],
                                r32(w1_sb[:, kc, m0:m1]),
                                r32(xT[:, kc, :]),
                                start=(kc == 0),
                                stop=(kc == 15),
                            )
                    for i in range(4):
                        pmc[half * 4 + i] = pp[i]

                # ---- q: nope chunks straight, rope chunk roped ----
                qfT = work.tile([128, 4, BLK], f32r, tag="qfT", bufs=2)
                nc.vector.tensor_copy(qfT[:, 0, :], pmc[0][:])
                nc.vector.tensor_copy(qfT[:, 1, :], pmc[1][:])
                cs, sn = csb[:], ssb[:]
                rot = work.tile([128, BLK], f32r, tag="rot", bufs=1)
                pq = pmc[2]
                # rotate_half pieces: 32-wide quadrant-aligned copies
                for hh in range(2):
                    r0 = hh * 64
                    nc.vector.tensor_scalar_mul(
                        rot[r0 : r0 + 32, :], pq[r0 + 32 : r0 + 64, :], -1.0
                    )
                    nc.vector.tensor_copy(rot[r0 + 32 : r0 + 64, :], pq[r0 : r0 + 32, :])
                nc.vector.tensor_mul(out=qfT[:, 2, :], in0=pq[:], in1=cs)
                nc.vector.tensor_mul(out=rot[:], in0=rot[:], in1=sn)
                nc.vector.tensor_add(out=qfT[:, 2, :], in0=qfT[:, 2, :], in1=rot[:])
                # h1 roped rows 64:128 -> chunk 3 rows 0:64 (32-wide aligned moves)
                nc.vector.tensor_copy(qfT[0:32, 3, :], qfT[64:96, 2, :])
                nc.vector.tensor_copy(qfT[32:64, 3, :], qfT[96:128, 2, :])

                # ---- latent chunks + sum of squares ----
                latent = work.tile([128, 4, BLK], f32r, tag="latent", bufs=1)
                ssq = ps.tile([1, BLK], f32, tag="st", bufs=2)
                for i in range(4):
                    pl = pmc[3 + i]
                    nc.vector.tensor_copy(latent[:, i, :], pl[:])
                    sqc = work.tile([128, BLK], f32r, tag="sqc", bufs=1)
                    nc.vector.tensor_mul(out=sqc[:], in0=latent[:, i, :], in1=pl[:])
                    nc.tensor.matmul(
                        ssq[:],
                        ones[:],
                        sqc[:],
                        start=(i == 0),
                        stop=(i == 3),
                    )

                # ---- krope: rope chunk 7 rows 0:64 ----
                pk = pmc[7]
                kr = krope[:, tok]
                nc.vector.tensor_scalar_mul(rot[0:32, :], pk[32:64, :], -1.0)
                nc.vector.tensor_copy(rot[32:64, :], pk[0:32, :])
                nc.vector.tensor_mul(out=kr, in0=pk[0:64, :], in1=cs[0:64, :])
                nc.vector.tensor_mul(out=rot[0:64, :], in0=rot[0:64, :], in1=sn[0:64, :])
                nc.vector.tensor_add(out=kr, in0=kr, in1=rot[0:64, :])

                # ---- rms scale: 1/sqrt(ssq/512 + eps), broadcast 128 wide ----
                lnrow = work.tile([1, BLK], f32, tag="lnrow", bufs=2)
                nc.scalar.activation(lnrow[:], ssq[:], LN, bias=eps[:], scale=1.0 / KVR)
                invrow = work.tile([1, BLK], f32r, tag="invrow", bufs=2)
                nc.scalar.activation(invrow[:], lnrow[:], EXP, scale=-0.5)
                invbc_ps = ps.tile([128, BLK], f32, tag="st", bufs=2, name="invbc_ps")
                nc.tensor.matmul(invbc_ps[:], onesrow[:], invrow[:])
                invbc = work.tile([128, BLK], f32, tag="invbc", bufs=1)
                nc.vector.tensor_copy(invbc[:], invbc_ps[:])

                # ---- kv up-projection + normalize;  v transposed to natural ----
                for mc in range(4):  # [h0 nope, h0 v, h1 nope, h1 v]
                    h = mc // 2
                    pkv = ps.tile([128, BLK], f32, tag="proj", bufs=4)
                    for kc in range(4):
                        nc.tensor.matmul(
                            pkv[:],
                            r32(wb_sb[:, kc, mc * 128 : (mc + 1) * 128]),
                            r32(latent[:, kc, :]),
                            start=(kc == 0),
                            stop=(kc == 3),
                        )
                    if mc % 2 == 0:
                        nc.vector.tensor_mul(
                            out=knope[h][:, tok], in0=pkv[:], in1=invbc[:]
                        )
                    else:
                        vuT = work.tile([128, BLK], f32r, tag="vuT", bufs=1)
                        nc.vector.tensor_mul(out=vuT[:], in0=pkv[:], in1=invbc[:])
                        pvt = ps.tile([128, BLK], f32, tag="xps", bufs=1)
                        for tt in range(4):
                            nc.tensor.transpose(
                                r32(pvt[:, tt * 128 : (tt + 1) * 128]),
                                r32(vuT[:, tt * 128 : (tt + 1) * 128]),
                                r32(ident[:]),
                            )
                        for tt in range(4):
                            nc.vector.tensor_copy(
                                vnat[h][:, qc * 4 + tt, :],
                                pvt[:, tt * 128 : (tt + 1) * 128],
                            )
            return qfT

        def stage_b(b, qc, qfT, knope, vnat, krope):
            """Causal attention for one q-chunk, both heads."""
            if True:
                n_kt = 4 * (qc + 1)
                for h in range(2):
                    yacc = ps.tile([VD, BLK], f32, tag="yacc", bufs=1)
                    acc = work.tile([128, BLK], f32r, tag="acc", bufs=1)
                    qrope = qfT[0:64, 2 + h, :]
                    for kt in range(n_kt):
                        ks = slice(kt * 128, (kt + 1) * 128)
                        st = ps.tile([128, BLK], f32, tag="st", bufs=2)
                        nc.tensor.matmul(
                            st[:],
                            r32(knope[h][:, ks]),
                            r32(qfT[:, h, :]),
                            start=True,
                            stop=False,
                        )
                        nc.tensor.matmul(
                            st[:],
                            r32(krope[:, ks]),
                            r32(qrope),
                            start=False,
                            stop=True,
                        )
                        est = work.tile([128, BLK], f32r, tag="est", bufs=2)
                        nc.scalar.activation(est[:], st[:], EXP)
                        if (kt + 1) * 128 > qc * BLK:
                            # zero out future positions:  keep k <= q
                            nc.gpsimd.affine_select(
                                out=est[:],
                                in_=est[:],
                                compare_op=mybir.AluOpType.is_ge,
                                fill=0.0,
                                base=qc * BLK - kt * 128,
                                pattern=[[1, BLK]],
                                channel_multiplier=-1,
                            )
                        nc.tensor.matmul(
                            yacc[:],
                            r32(vnat[h][:, kt, :]),
                            r32(est[:]),
                            start=(kt == 0),
                            stop=(kt == n_kt - 1),
                        )
                        if kt == 0:
                            nc.gpsimd.tensor_copy(acc[:], est[:])
                        else:
                            nc.gpsimd.tensor_add(out=acc[:], in0=acc[:], in1=est[:])

                    sums = ps.tile([1, BLK], f32, tag="st", bufs=2)
                    nc.tensor.matmul(sums[:], ones[:], acc[:])
                    lnr = work.tile([1, BLK], f32, tag="lnrow", bufs=2)
                    nc.scalar.activation(lnr[:], sums[:], LN)
                    sinvrow = work.tile([1, BLK], f32r, tag="invrow", bufs=2)
                    nc.scalar.activation(sinvrow[:], lnr[:], EXP, scale=-1.0)
                    sbc_ps = ps.tile([128, BLK], f32, tag="st", bufs=2, name="sbc_ps")
                    nc.tensor.matmul(sbc_ps[:], onesrow[:], sinvrow[:])
                    sinv = work.tile([128, BLK], f32, tag="sinv", bufs=1)
                    nc.vector.tensor_copy(sinv[:], sbc_ps[:])
                    ysb = work.tile([VD, BLK], f32, tag="ysb", bufs=2)
                    nc.vector.tensor_mul(out=ysb[:], in0=yacc[:], in1=sinv[:])
                    for jj in range(2):
                        nc.sync.dma_start(
                            y_in[b][qc * 2 + jj, h * VD : (h + 1) * VD, :],
                            ysb[:, jj * 256 : (jj + 1) * 256],
                        )

        # software pipeline: emit projections of block qc+1 interleaved with
        # attention of block qc so the PE always has dense independent work
        for b in range(B):
            knope = [
                kvpool.tile([NOPE, T], f32r, tag=f"knope{h}", bufs=1, name=f"knope{h}")
                for h in range(2)
            ]
            vnat = [
                kvpool.tile([128, 16, VD], f32r, tag=f"vnat{h}", bufs=1, name=f"vnat{h}")
                for h in range(2)
            ]
            krope = kvpool.tile([ROPE, T], f32r, tag="krope", bufs=1)

            prev_qfT = None
            for qc in range(4):
                cur_qfT = stage_a(b, qc, knope, vnat, krope)
                if prev_qfT is not None:
                    stage_b(b, qc - 1, prev_qfT, knope, vnat, krope)
                prev_qfT = cur_qfT
            stage_b(b, 3, prev_qfT, knope, vnat, krope)

            # exchange this batch's outputs; batch-0 A2A overlaps batch-1 work
            nc.gpsimd.collective_compute(
                "AllToAll",
                mybir.AluOpType.bypass,
                replica_groups=[list(range(NCORES))],
                ins=[y_in[b].opt()],
                outs=[y_out[b].opt()],
            )


def _phase2_wo(nc, tc, y_out, wo_d, out_d):
    with (
        tc.tile_pool(name="wopool", bufs=1) as wop,
        tc.tile_pool(name="ps2", bufs=1, space="PSUM") as ps2,
    ):
        a2a = []
        for b in range(B):
            t = wop.tile([128, 16, 256], f32r, tag=f"a2a{b}", name=f"a2a{b}")
            for kc in range(16):
                nc.sync.dma_start(
                    t[:, kc, :],
                    y_out[b][kc // 2, (kc % 2) * 128 : (kc % 2) * 128 + 128, :]
                    .bitcast(f32r),
                )
            a2a.append(t)
        for n in range(4):
            pouts = [
                ps2.tile([128, 512], f32, tag="outp", bufs=4, name="outp")
                for _ in range(4)
            ]
            for kc in range(16):
                wt = wop.tile([128, 512], f32r, tag="wt", bufs=4)
                nc.sync.dma_start(
                    wt[:],
                    wo_d[kc * 128 : (kc + 1) * 128, n * 512 : (n + 1) * 512].bitcast(f32r),
                )
                for b in range(B):
                    for tt in range(2):
                        nc.tensor.matmul(
                            pouts[b * 2 + tt][:],
                            r32(a2a[b][:, kc, tt * 128 : (tt + 1) * 128]),
                            r32(wt[:]),
                            start=(kc == 0),
                            stop=(kc == 15),
                        )
            for b in range(B):
                for tt in range(2):
                    osb = wop.tile([128, 512], f32, tag="osb", bufs=4)
                    nc.vector.tensor_copy(osb[:], pouts[b * 2 + tt][:])
                    nc.sync.dma_start(
                        out_d[b, tt * 128 : (tt + 1) * 128, n * 512 : (n + 1) * 512],
                        osb[:],
                    )


def host_prep(x, wq, wkv_a, wkv_b, wo, kv_norm_w):
    scale = np.float32(QKD ** -0.5)
    inv = (1.0 / (10000.0 ** (np.arange(0, ROPE, 2, dtype=np.float32) / ROPE))).astype(
        np.float32
    )
    f = np.outer(np.arange(T, dtype=np.float32), inv)
    cos32 = np.cos(f).T.astype(np.float32)
    sin32 = np.sin(f).T.astype(np.float32)
    cos128 = np.ascontiguousarray(np.concatenate([cos32] * 4, 0))
    sin128 = np.ascontiguousarray(np.concatenate([sin32] * 4, 0))
    wkv_bw = (wkv_b * kv_norm_w[:, None]).astype(np.float32)
    x2 = np.ascontiguousarray(x.reshape(B * T, D))
    wo_c = np.ascontiguousarray(wo)
    wq_r = wq.reshape(D, H, QKD)

    in_maps = []
    for c in range(NCORES):
        h0 = HPC * c
        w1 = np.concatenate(
            [
                wq_r[:, h0, :NOPE] * scale,
                wq_r[:, h0 + 1, :NOPE] * scale,
                wq_r[:, h0, NOPE:] * scale,
                wq_r[:, h0 + 1, NOPE:] * scale,
                wkv_a,
            ],
            axis=1,
        ).astype(np.float32)
        wb = np.ascontiguousarray(
            wkv_bw[:, h0 * (NOPE + VD) : (h0 + 2) * (NOPE + VD)]
        )
        in_maps.append(
            {
                "x": x2,
                "w1": np.ascontiguousarray(w1),
                "wb": wb,
                "wo": wo_c,
                "cos": cos128,
                "sin": sin128,
            }
        )
    return in_maps


_NC = None


def kernel(x, wq, wkv_a, wkv_b, wo, kv_norm_w, _trace=False):
    global _NC
    if _NC is None:
        _NC = build_program()
    in_maps = host_prep(
        np.asarray(x, np.float32),
        np.asarray(wq, np.float32),
        np.asarray(wkv_a, np.float32),
        np.asarray(wkv_b, np.float32),
        np.asarray(wo, np.float32),
        np.asarray(kv_norm_w, np.float32),
    )
    res = run_bass_kernel_spmd(_NC, in_maps, list(range(NCORES)), trace=_trace)
    out = np.empty((B, T, D), np.float32)
    cw = T // NCORES
    for c in range(NCORES):
        oc = res.results[c]["out"]  # (B, 256, D)
        for b in range(B):
            out[b, c * cw : (c + 1) * cw, :] = oc[b]
    kernel.last_results = res
    return out
